# revision 18
# baseline (speedup 1.0000x reference)
"""Bidirectional GRU (Keras reset_after=True) Trainium2 Bass kernel.

Problem shapes: B=32, T=256, D=1024, H=1024 (3H=3072), fp32.

Sharding (8 cores, uniform SPMD program, per-core data differs):
  - cores 0..3: forward direction, batch quarters 0..3 (Bl=8 each)
  - cores 4..7: backward direction, batch quarters 0..3, with x time-reversed
    on the host; outputs are time-flipped back on the host.
Each core runs: input projection xg = x @ Wk + b (PE, bf16), the full
T-step recurrence for its direction/batch-quarter (PE LDW-bound, bf16
weights, f32 state + PSUM accumulation), and the output transpose
[H,(t,b)] -> [b,t,H] (PE transpose, f32).

Host only shards inputs (slice + time-flip for the backward cores) and
concatenates per-core outputs.
"""

import numpy as np
import jax

# Persistent XLA/NEFF compile cache — makes repeat invocations skip the
# multi-minute neuronxcc compile.
try:
    jax.config.update("jax_compilation_cache_dir", "/tmp/jax_neff_cache")
    jax.config.update("jax_persistent_cache_min_entry_size_bytes", -1)
    jax.config.update("jax_persistent_cache_min_compile_time_secs", 0.0)
except Exception:
    pass

import concourse.bass as bass
import concourse.bacc as bacc
import concourse.mybir as mybir
from concourse.tile import TileContext
from concourse.bass_utils import run_bass_kernel_spmd

F32 = mybir.dt.float32
BF16 = mybir.dt.bfloat16

B, T, D, H = 32, 256, 1024, 1024
G3 = 3 * H                      # 3072
NCORES = 8
BL = B // (NCORES // 2)         # 8 batch rows per core
KD = D // 128                   # 8 k-tiles over D
KH = H // 128                   # 8 k-tiles over H
M3 = G3 // 128                  # 24 m-tiles over 3H
TBLK = 16                       # recurrence xg prefetch block (time steps)


def build_nc(nt: int = T):
    """Build the per-core Bass program for nt time steps (nt % TBLK == 0)."""
    assert nt % TBLK == 0
    ntb = nt * BL               # rows of x per core

    nc = bacc.Bacc("TRN2", target_bir_lowering=False, debug=False,
                   num_devices=NCORES)

    x = nc.dram_tensor("x", [BL, nt, D], F32, kind="ExternalInput")
    wk = nc.dram_tensor("wk", [D, G3], F32, kind="ExternalInput")
    wr = nc.dram_tensor("wr", [H, G3], F32, kind="ExternalInput")
    bias = nc.dram_tensor("bias", [2, G3], F32, kind="ExternalInput")
    ident = nc.dram_tensor("ident", [128, 128], F32, kind="ExternalInput")
    out = nc.dram_tensor("out", [BL, nt, H], F32, kind="ExternalOutput")

    # internal DRAM scratch
    xgT_d = nc.dram_tensor("xgT_d", [M3, 128, nt, BL], F32)   # proj, transposed
    hT_d = nc.dram_tensor("hT_d", [KH, 128, nt, BL], F32)     # h history

    with TileContext(nc) as tc:
        # ---------- persistent pools ----------
        with (
            tc.tile_pool(name="rkw", bufs=1) as rkw_pool,
            tc.tile_pool(name="consts", bufs=1) as const_pool,
            tc.tile_pool(name="ld", bufs=3) as ld_pool,
        ):
            # rkernel -> bf16, resident whole kernel lifetime [128, KH*G3]
            rk_bf = rkw_pool.tile([128, KH * G3], BF16, bufs=1)
            for k in range(KH):
                w32 = ld_pool.tile([128, G3], F32, tag="wslab", bufs=2)
                nc.sync.dma_start(out=w32[:], in_=wr.ap()[k * 128:(k + 1) * 128, :])
                nc.vector.tensor_copy(rk_bf[:, k * G3:(k + 1) * G3], w32[:])

            # identity (f32 + bf16 copies) for PE transposes
            id32 = const_pool.tile([128, 128], F32)
            nc.sync.dma_start(out=id32[:], in_=ident.ap())
            idbf = const_pool.tile([128, 128], BF16)
            nc.vector.tensor_copy(idbf[:], id32[:])

            # bias prep:
            #  bias_fold[p, m] = b_in[m*128+p] + (b_rec[...] for z,r gates)
            #  bh[p, hc]      = b_rec_h[hc*128+p]
            bin_t = const_pool.tile([128, M3], F32)
            nc.sync.dma_start(out=bin_t[:],
                              in_=bias.ap()[0, :].rearrange("(m p) -> p m", p=128))
            brzr_t = const_pool.tile([128, 2 * KH], F32)
            nc.sync.dma_start(out=brzr_t[:],
                              in_=bias.ap()[1, 0:2 * H].rearrange("(m p) -> p m", p=128))
            bias_fold = const_pool.tile([128, M3], F32)
            nc.vector.tensor_add(bias_fold[:, 0:2 * KH], bin_t[:, 0:2 * KH], brzr_t[:])
            nc.vector.tensor_copy(bias_fold[:, 2 * KH:M3], bin_t[:, 2 * KH:M3])
            bh_t = const_pool.tile([128, KH], F32)
            nc.sync.dma_start(out=bh_t[:],
                              in_=bias.ap()[1, 2 * H:3 * H].rearrange("(c p) -> p c", p=128))

            # ---------- stage A2 + B: x transpose and input projection ----------
            with (
                tc.tile_pool(name="proj", bufs=2) as proj_pool,
                tc.tile_pool(name="projp", bufs=2, space="PSUM") as projp_pool,
            ):
                # kernel -> bf16 [128, KD*G3]
                kb_bf = proj_pool.tile([128, KD * G3], BF16, tag="kb", bufs=1)
                for k in range(KD):
                    w32 = ld_pool.tile([128, G3], F32, tag="wslab", bufs=2)
                    nc.sync.dma_start(out=w32[:], in_=wk.ap()[k * 128:(k + 1) * 128, :])
                    nc.vector.tensor_copy(kb_bf[:, k * G3:(k + 1) * G3], w32[:])

                # xT bf16 [128, KD * ntb], free order (t, b) t-major
                xT_bf = proj_pool.tile([128, KD * ntb], BF16, tag="xT", bufs=1)
                # x rows regrouped: partition=(t_local, b) [16t x 8b], free=d-chunk
                x_r = x.ap().rearrange("b (c tl) (k dj) -> c k tl b dj",
                                       tl=TBLK, dj=128)
                nck = nt // TBLK
                for c in range(nck):
                    for k in range(KD):
                        x32 = ld_pool.tile([128, 128], F32, tag="x32")
                        nc.sync.dma_start(out=x32[:], in_=x_r[c, k])
                        xbf = ld_pool.tile([128, 128], BF16, tag="xbf")
                        nc.vector.tensor_copy(xbf[:], x32[:])
                        ps = projp_pool.tile([128, 128], BF16, tag="tps")
                        nc.tensor.transpose(ps[:], xbf[:], idbf[:])
                        nc.vector.tensor_copy(
                            xT_bf[:, k * ntb + c * 128:k * ntb + (c + 1) * 128], ps[:])

                # projection: xgT_d[m, p, t, b] = (x @ wk + bias_fold)^T
                NCH = max(1, ntb // 512)     # 512-wide psum chunks
                CW = min(512, ntb)
                for m in range(M3):
                    xg_s = proj_pool.tile([128, ntb], F32, tag="xgs")
                    for c4 in range(NCH):
                        pp = projp_pool.tile([128, CW], F32, tag="pp")
                        for k in range(KD):
                            nc.tensor.matmul(
                                pp[:],
                                kb_bf[:, k * G3 + m * 128:k * G3 + (m + 1) * 128],
                                xT_bf[:, k * ntb + c4 * CW:k * ntb + (c4 + 1) * CW],
                                start=(k == 0), stop=(k == KD - 1))
                        nc.scalar.activation(
                            xg_s[:, c4 * CW:(c4 + 1) * CW], pp[:],
                            mybir.ActivationFunctionType.Identity,
                            bias=bias_fold[:, m:m + 1])
                    nc.sync.dma_start(
                        out=xgT_d.ap()[m].rearrange("p t b -> p (t b)"),
                        in_=xg_s[:])

            # ---------- stage C: recurrence ----------
            with (
                tc.tile_pool(name="rec", bufs=2) as rec_pool,
                tc.tile_pool(name="st", bufs=1) as st_pool,
                tc.tile_pool(name="recp", bufs=2, space="PSUM") as recp_pool,
            ):
                # persistent state: f32 master h + bf16 matmul copy, 2 halves
                h32 = [st_pool.tile([128, 4, BL], F32, tag=f"h32_{i}",
                                    name=f"h32_{i}") for i in (0, 1)]
                hbf = [st_pool.tile([128, 4, BL], BF16, tag=f"hbf_{i}",
                                    name=f"hbf_{i}") for i in (0, 1)]
                for i in (0, 1):
                    nc.gpsimd.memset(h32[i][:], 0.0)
                    nc.gpsimd.memset(hbf[i][:], 0.0)

                xg_r = xgT_d.ap().rearrange("m p t b -> p m t b")
                hT_r = hT_d.ap().rearrange("hc p t b -> p hc t b")
                # half -> list of m-tiles (chunks 0..3 / 4..7 of each gate)
                half_ms = [[g * KH + c for g in range(3) for c in range(4)],
                           [g * KH + 4 + c for g in range(3) for c in range(4)]]

                loop_ctx = tc.For_i(0, nt, TBLK,
                                    hint_engines=(mybir.EngineType.PE,))
                with loop_ctx as tb0:
                    xgblk = rec_pool.tile([128, M3, TBLK, BL], F32, tag="xgblk")
                    nc.sync.dma_start(
                        out=xgblk[:],
                        in_=xg_r[:, :, bass.ds(tb0, TBLK), :])

                    for toff in range(TBLK):
                        psum = [recp_pool.tile([128, 12 * BL], F32, tag=f"ps{i}",
                                               name=f"ps{i}") for i in (0, 1)]
                        # PE: half A (m-chunks 0..3 of each gate) then half B;
                        # k contiguous 0..7 per m-slice (PSUM group constraint)
                        for mh in (0, 1):
                            for mi, m in enumerate(half_ms[mh]):
                                for k in range(KH):
                                    kh2, kc = divmod(k, 4)
                                    nc.tensor.matmul(
                                        psum[mh][:, mi * BL:(mi + 1) * BL],
                                        rk_bf[:, k * G3 + m * 128:
                                              k * G3 + (m + 1) * 128],
                                        hbf[kh2][:].rearrange("p c b -> p (c b)")[
                                            :, kc * BL:(kc + 1) * BL],
                                        start=(k == 0), stop=(k == KH - 1))

                        # gate math per half (chunks hh*4 .. hh*4+3)
                        for hh2 in (0, 1):
                            pv = psum[hh2][:].rearrange("p (g c b) -> p g c b",
                                                        g=3, c=4)
                            xz = xgblk[:, hh2 * 4 + 0:hh2 * 4 + 4, toff, :]
                            xr = xgblk[:, KH + hh2 * 4: KH + hh2 * 4 + 4, toff, :]
                            xh = xgblk[:, 2 * KH + hh2 * 4: 2 * KH + hh2 * 4 + 4,
                                       toff, :]

                            zpre = rec_pool.tile([128, 4, BL], F32, tag=f"zpre{hh2}")
                            nc.vector.tensor_add(zpre[:], pv[:, 0], xz)
                            z = rec_pool.tile([128, 4, BL], F32, tag=f"z{hh2}")
                            nc.scalar.activation(z[:], zpre[:],
                                                 mybir.ActivationFunctionType.Sigmoid)
                            rpre = rec_pool.tile([128, 4, BL], F32, tag=f"rpre{hh2}")
                            nc.vector.tensor_add(rpre[:], pv[:, 1], xr)
                            r = rec_pool.tile([128, 4, BL], F32, tag=f"r{hh2}")
                            nc.scalar.activation(r[:], rpre[:],
                                                 mybir.ActivationFunctionType.Sigmoid)
                            rhb = rec_pool.tile([128, 4, BL], F32, tag=f"rhb{hh2}")
                            nc.vector.tensor_add(
                                rhb[:], pv[:, 2],
                                bh_t[:, hh2 * 4:hh2 * 4 + 4, None].to_broadcast(
                                    (128, 4, BL)))
                            hpre = rec_pool.tile([128, 4, BL], F32, tag=f"hpre{hh2}")
                            nc.vector.tensor_mul(hpre[:], r[:], rhb[:])
                            nc.vector.tensor_add(hpre[:], hpre[:], xh)
                            hcand = rec_pool.tile([128, 4, BL], F32, tag=f"hc{hh2}")
                            nc.scalar.activation(hcand[:], hpre[:],
                                                 mybir.ActivationFunctionType.Tanh)
                            # h = hcand + z*(h - hcand)
                            dtl = rec_pool.tile([128, 4, BL], F32, tag=f"d{hh2}")
                            nc.vector.tensor_sub(dtl[:], h32[hh2][:], hcand[:])
                            nc.vector.tensor_mul(dtl[:], z[:], dtl[:])
                            nc.vector.tensor_add(h32[hh2][:], hcand[:], dtl[:])
                            nc.vector.tensor_copy(hbf[hh2][:], h32[hh2][:])
                            nc.sync.dma_start(
                                out=hT_r[:, hh2 * 4:hh2 * 4 + 4,
                                         bass.ds(tb0 + toff, 1), :],
                                in_=h32[hh2][:])

            # ---------- stage D: output transpose ----------
            with (
                tc.tile_pool(name="outp", bufs=3) as out_pool,
                tc.tile_pool(name="outps", bufs=2, space="PSUM") as outps_pool,
            ):
                out_r = out.ap().rearrange("b (c tl) (hc hj) -> hc c tl b hj",
                                           tl=TBLK, hj=128)
                ncols = nt * BL // 128
                for hc in range(KH):
                    for c in range(ncols):
                        hts = out_pool.tile([128, 128], F32, tag="hts")
                        nc.sync.dma_start(
                            out=hts[:],
                            in_=hT_d.ap()[hc, :, c * TBLK:(c + 1) * TBLK, :]
                                .rearrange("p t b -> p (t b)"))
                        tps = outps_pool.tile([128, 128], F32, tag="ops")
                        nc.tensor.transpose(tps[:], hts[:], id32[:])
                        ots = out_pool.tile([128, 128], F32, tag="ots")
                        nc.vector.tensor_copy(ots[:], tps[:])
                        nc.sync.dma_start(out=out_r[hc, c], in_=ots[:])

    nc.compile()
    return nc


_NC_CACHE = {}


def _get_nc(nt: int):
    if nt not in _NC_CACHE:
        _NC_CACHE[nt] = build_nc(nt)
    return _NC_CACHE[nt]


def make_in_maps(inputs, kernel_f, rkernel_f, bias_f, kernel_b, rkernel_b,
                 bias_b):
    eye = np.eye(128, dtype=np.float32)
    f32 = np.float32
    in_maps = []
    for q in range(4):
        in_maps.append({
            "x": np.ascontiguousarray(inputs[q * BL:(q + 1) * BL], dtype=f32),
            "wk": np.asarray(kernel_f, dtype=f32),
            "wr": np.asarray(rkernel_f, dtype=f32),
            "bias": np.asarray(bias_f, dtype=f32),
            "ident": eye,
        })
    for q in range(4):
        in_maps.append({
            "x": np.ascontiguousarray(
                inputs[q * BL:(q + 1) * BL][:, ::-1], dtype=f32),
            "wk": np.asarray(kernel_b, dtype=f32),
            "wr": np.asarray(rkernel_b, dtype=f32),
            "bias": np.asarray(bias_b, dtype=f32),
            "ident": eye,
        })
    return in_maps


def assemble(results, nt: int = T):
    outputs = np.empty((B, nt, 2 * H), dtype=np.float32)
    for q in range(4):
        outputs[q * BL:(q + 1) * BL, :, :H] = results[q]["out"]
        outputs[q * BL:(q + 1) * BL, :, H:] = results[4 + q]["out"][:, ::-1]
    state = np.concatenate([outputs[:, -1, :H], outputs[:, 0, H:]], axis=1)
    return outputs, state


def kernel(inputs, kernel_f, rkernel_f, bias_f, kernel_b, rkernel_b, bias_b):
    nc = _get_nc(T)
    in_maps = make_in_maps(np.asarray(inputs), kernel_f, rkernel_f, bias_f,
                           kernel_b, rkernel_b, bias_b)
    res = run_bass_kernel_spmd(nc, in_maps, list(range(NCORES)))
    return assemble(res.results)
